# revision 65
# baseline (speedup 1.0000x reference)
"""Multi-head causal attention (B=2, S=2048, D=1024, H=16, hd=64) on 8 trn2 cores.

Sharding: core c handles batch b = c//4 and head-group g = c%4 (heads 4g..4g+4,
d-slice 256g..256g+256 of the QKV projections / Wo rows).  Each core computes a
partial out-projection [2048, 1024] in bf16; the host sums the 4 head-group
partials per batch in f32 and adds the bias.

Per-core kernel (all matmuls bf16, accumulate f32 in PSUM):
  qT/kT = (x @ Wq/k)^T computed directly as [256, 2048] via lhsT=W chunks.
  v     = x @ Wv in natural [seq, head, 66] layout (col 64 = 1.0 so the
          attention rowsum falls out of the ctx matmul; col 65 = 0 pad).
  S^T   = k_h @ q_h^T  [kpos, qpos] tiles, both heads of a pair concurrently
          via PE row tiling; exp via ACT (scale=1/8) PSUM->SBUF; causal = skip
          invalid column blocks + triangular bf16 mask on diagonal blocks.
  ctx~T = v'_h^T @ expS^T accumulated over kpos blocks -> [66, 512] PSUM
          (row 64 = softmax denominator).
  norm:  rowsums of both heads copied into one [2,512] tile (ACT+DVE), one
         fast-reciprocal, bf16 downcast, then a K=2 PE matmul against a
         selector constant broadcasts 1/rowsum to [128, 512] PSUM; DVE
         multiplies yield normalized ctx~T (no DRAM bounce round trips).
  out  += ctx~T^T @ Wo rows, stored bf16 per 128-row block as soon as its
         column group's attention is done.

Schedule: ascending q-column groups with just-in-time projections: v/qk(j=0)
projections first, then per group g: attention(pair0, g), attention(pair1, g)
with later projections and the previous group's out-projection interleaved as
PE filler between kb steps.  The attention kb loop is software-pipelined
(scores(kb+1) and a filler are emitted between exp(kb) and ctx(kb)) so the
in-order PE never waits on ACT.  Normalization part B (PE broadcast + DVE
multiplies) is deferred into the next group-half's filler stream, hiding the
reciprocal chain latency.  The final four out-proj blocks cycle their PSUM
chains across all three pools (free once attention ends) to hide copy drain.
Inputs arrive as staged DMAs (weights prepacked host-side into [128, 2048]
row-major tiles; x in column quarters) so the PE starts ~12us in; output is
stored bf16 in half-block stores spread across the back half of the kernel.
"""

import sys

import numpy as np

for _p in ("/opt/trn_rl_repo",):
    if _p not in sys.path:
        sys.path.insert(0, _p)

import ml_dtypes

import concourse.bass as bass
import concourse.mybir as mybir
import concourse.tile as tile
from concourse import bacc
from concourse.bass_utils import run_bass_kernel_spmd
from concourse.masks import make_upper_triangular

BF16 = mybir.dt.bfloat16
F32 = mybir.dt.float32

B, S, D, H, HD = 2, 2048, 1024, 16, 64
NCORES = 8
HPC = 4          # heads per core
DHC = HPC * HD   # 256: d-slice per core
P = 128
SB = S // P      # 16 seq blocks
KC = D // P      # 8 contraction chunks for projections
QG = 512         # q column group width
NQG = S // QG    # 4
VW = HD + 2      # 66: v cols per head (64 data + ones + pad; even M for PE)


def _build_body(ctx, tc, io):
    nc = tc.nc
    xT, wq, wk, wv, wo, out = (
        io["xT"], io["wq"], io["wk"], io["wv"], io["wo"], io["out"],
    )

    consts = ctx.enter_context(tc.tile_pool(name="consts", bufs=1))
    persist = ctx.enter_context(tc.tile_pool(name="persist", bufs=1))
    spool = ctx.enter_context(tc.tile_pool(name="spsum", bufs=2, space="PSUM"))
    cxpool = ctx.enter_context(tc.tile_pool(name="cxpsum", bufs=3, space="PSUM"))
    pjpool = ctx.enter_context(tc.tile_pool(name="pjpsum", bufs=1, space="PSUM"))
    espool = ctx.enter_context(tc.tile_pool(name="es", bufs=6))
    nrmpool = ctx.enter_context(tc.tile_pool(name="nrm", bufs=4))
    outpool = ctx.enter_context(tc.tile_pool(name="outsb", bufs=3))

    # triangular keep-mask for diagonal blocks: tri[i, j] = 1.0 iff j >= i
    tri = consts.tile([P, P], BF16, tag="tri", name="tri")
    make_upper_triangular(nc, tri[:], val=1.0, diag=True)
    # tri[0:1, 0:HD] doubles as the all-ones [1, 64] vector for K=1
    # partition-broadcast matmuls in the softmax normalization.

    # ---- input tiles + staged DMA issue order ----
    wq_sb = persist.tile([P, KC, DHC], BF16, tag="wq", name="wq")
    wk_sb = persist.tile([P, KC, DHC], BF16, tag="wk", name="wk")
    wv_sb = persist.tile([P, KC, DHC], BF16, tag="wv", name="wv")
    wo_sb = persist.tile([P, 2, D], BF16, tag="wo", name="wo")
    xt = [persist.tile([P, S], BF16, tag=f"xt{k}", name=f"xt{k}")
          for k in range(KC)]

    HK = KC // 2  # weight half: 4 k-chunks
    HX = S // 2   # x column half
    QX = HX // 2  # x column quarter

    def dma_x(k, q):
        nc.sync.dma_start(out=xt[k][:, q * QX:(q + 1) * QX],
                          in_=xT[k * P:(k + 1) * P, q * QX:(q + 1) * QX])

    # interleave wv chunk-pairs with the x chunks they gate so the first
    # v-projection's 8-chunk PSUM chain starts after ~3 DMAs land
    for h in range(4):
        nc.sync.dma_start(out=wv_sb[:, 2 * h:2 * (h + 1), :],
                          in_=wv[:, 2 * h * DHC:2 * (h + 1) * DHC])
        dma_x(2 * h, 0)
        dma_x(2 * h + 1, 0)
    for w_sb, dram in ((wq_sb, wq), (wk_sb, wk)):
        for h in range(2):
            nc.sync.dma_start(out=w_sb[:, h * HK:(h + 1) * HK, :],
                              in_=dram[:, h * HK * DHC:(h + 1) * HK * DHC])
    for k in range(KC):
        dma_x(k, 1)
    for q in range(2, 4):
        for k in range(KC):
            dma_x(k, q)
    for h in range(2):
        nc.sync.dma_start(out=wo_sb[:, h, :], in_=wo[:, h * D:(h + 1) * D])

    # persistent tensors
    v_sb = [persist.tile([P, HPC, VW], BF16, tag=f"v{s}", name=f"v{s}")
            for s in range(SB)]
    qt = [persist.tile([P, S], BF16, tag=f"qt{i}", name=f"qt{i}") for i in range(2)]
    kt = [persist.tile([P, S], BF16, tag=f"kt{i}", name=f"kt{i}") for i in range(2)]
    ctxT = [persist.tile([P, S], BF16, tag=f"ctxT{i}", name=f"ctxT{i}")
            for i in range(2)]

    # ---- emission helpers ----
    def emit_v_proj(sv):
        # two seq blocks (2*sv, 2*sv+1) -> v natural layout
        ps = spool.tile([P, 2, QG], F32, tag="sp", name="sp")
        for par in range(2):
            s = 2 * sv + par
            for k in range(KC):
                nc.tensor.matmul(
                    ps[:, par, 0:DHC],
                    lhsT=xt[k][:, s * P:(s + 1) * P],
                    rhs=wv_sb[:, k, :],
                    start=(k == 0),
                    stop=(k == KC - 1),
                )
            src_ap = ps[:, par, 0:DHC].rearrange("p (h d) -> p h d", h=HPC)
            nc.vector.tensor_copy(v_sb[s][:, :, 0:HD], src_ap)
            nc.vector.memset(v_sb[s][:, :, HD:VW], 1.0)
            nc.vector.memset(v_sb[s][:, :, HD + 1:VW], 0.0)

    def emit_qk_proj(pair, j):
        # q and k projections for d-chunk `pair`, q column group j
        for w_sb, dst in ((wq_sb, qt), (wk_sb, kt)):
            ps = pjpool.tile([P, QG], F32, tag="pj", name="pj")
            for k in range(KC):
                nc.tensor.matmul(
                    ps[:],
                    lhsT=w_sb[:, k, pair * P:(pair + 1) * P],
                    rhs=xt[k][:, j * QG:(j + 1) * QG],
                    start=(k == 0),
                    stop=(k == KC - 1),
                )
            nc.vector.tensor_copy(dst[pair][:, j * QG:(j + 1) * QG], ps[:])

    def emit_attention_group(pair, g, fillers):
        # fillers: list of zero-arg closures; one is popped and emitted after
        # each kb step to keep independent PE work between dependent steps.
        # The kb loop is software-pipelined: scores(kb+1) is emitted before
        # ctx(kb) so the PE runs scores while ACT computes exp(kb).
        cxs = [cxpool.tile([VW, QG], F32, tag="cx", name="cx") for _ in range(2)]
        nkb = 4 * g + 4

        def c0_of(kb):
            return P * (kb - 4 * g) if kb >= 4 * g else 0

        def emit_scores(kb):
            c0 = c0_of(kb)
            sp_t = spool.tile([P, 2, QG], F32, tag="sp", name="sp")
            for hh in range(2):
                nc.tensor.matmul(
                    sp_t[:, hh, c0:QG],
                    lhsT=kt[pair][hh * HD:(hh + 1) * HD, kb * P:(kb + 1) * P],
                    rhs=qt[pair][hh * HD:(hh + 1) * HD, g * QG + c0:(g + 1) * QG],
                    start=True,
                    stop=True,
                )
            return sp_t

        sp_next = emit_scores(0)
        for kb in range(nkb):
            c0 = c0_of(kb)
            sp_t = sp_next
            es_t = espool.tile([P, 2, QG], BF16, tag="es", name="es")
            nc.scalar.activation(
                es_t[:, :, c0:QG], sp_t[:, :, c0:QG],
                mybir.ActivationFunctionType.Exp, scale=0.125,
            )
            if kb + 1 < nkb:
                sp_next = emit_scores(kb + 1)
            if fillers:
                fillers.pop(0)()
            if kb == 0 and fillers:
                # second filler before the first ctx: covers the deferred
                # normalization's reciprocal-chain latency so ctx(0) doesn't
                # stall on the PSUM bank its predecessor frees
                fillers.pop(0)()
            if kb >= 4 * g:
                dst = es_t[:, :, c0:c0 + P]
                t_ap = tri[:]
                tri_b = bass.AP(t_ap.tensor, t_ap.offset,
                                [t_ap.ap[0], [0, 2], t_ap.ap[1]])
                nc.vector.tensor_mul(dst, dst, tri_b)
            for hh in range(2):
                h = 2 * pair + hh
                nc.tensor.matmul(
                    cxs[hh][:, c0:QG],
                    lhsT=v_sb[kb][:, h, :],
                    rhs=es_t[:, hh, c0:QG],
                    start=(kb == 0),
                    stop=(kb == nkb - 1),
                )
        while fillers:
            fillers.pop(0)()
        # softmax normalization part A (inline): per-head rowsum -> reciprocal
        # -> bf16 downcast, all on DVE ([1, QG] ops pipeline per head and the
        # ACT queue stays clear for the next group's exps).  Separate
        # offset-0 tiles per head: custom DVE ops need zero-offset APs.
        rcbs = []
        for hh in range(2):
            rs1 = nrmpool.tile([1, QG], F32, tag=f"rs{hh}", name="rs")
            rc1 = nrmpool.tile([1, QG], F32, tag=f"rc{hh}", name="rc")
            rcb1 = nrmpool.tile([1, QG], BF16, tag=f"rcb{hh}", name="rcb")
            nc.vector.tensor_copy(rs1[:], cxs[hh][HD:HD + 1, :])
            nc.vector.reciprocal_approx_fast(rc1[:], rs1[:])
            nc.vector.tensor_copy(rcb1[:], rc1[:])
            rcbs.append(rcb1)

        def norm_b():
            # part B (deferred into the next group's PE stream): K=1 PE
            # broadcasts of 1/rowsum + normalized fp-copy of ctx~T
            rb = pjpool.tile([P, QG], F32, tag="pj", name="rb")
            for hh in range(2):
                nc.tensor.matmul(
                    rb[hh * HD:(hh + 1) * HD, :],
                    lhsT=tri[0:1, 0:HD],
                    rhs=rcbs[hh][0:1, :],
                    start=True,
                    stop=True,
                )
            rbs = nrmpool.tile([P, QG], F32, tag="rbs", name="rbs")
            nc.vector.tensor_copy(rbs[:], rb[:])
            for hh in range(2):
                nc.vector.tensor_mul(
                    ctxT[pair][hh * HD:(hh + 1) * HD, g * QG:(g + 1) * QG],
                    cxs[hh][0:HD, :],
                    rbs[hh * HD:(hh + 1) * HD, :],
                )

        return norm_b

    def emit_outproj(m, pools=None):
        # pools: optional per-half psum pool/tag overrides; the final blocks
        # cycle over all pools (free once attention is done) so consecutive
        # chains don't serialize on one bank's copy drain.
        ot = outpool.tile([P, D], BF16, tag="ot", name="ot")
        for n2 in range(2):
            if pools is None:
                ps = pjpool.tile([P, QG], F32, tag="pj", name="pj")
            else:
                pool, tag = pools[n2]
                ps = pool.tile([P, QG], F32, tag=tag, name="pj")
            for kc in range(2):
                nc.tensor.matmul(
                    ps[:],
                    lhsT=ctxT[kc][:, m * P:(m + 1) * P],
                    rhs=wo_sb[:, kc, n2 * QG:(n2 + 1) * QG],
                    start=(kc == 0),
                    stop=(kc == 1),
                )
            if pools is None:
                nc.vector.tensor_copy(ot[:, n2 * QG:(n2 + 1) * QG], ps[:])
            else:
                # final blocks: ACT is idle at the tail, keep DVE clear
                nc.scalar.copy(ot[:, n2 * QG:(n2 + 1) * QG], ps[:])
            nc.sync.dma_start(
                out=out[m * P:(m + 1) * P, n2 * QG:(n2 + 1) * QG],
                in_=ot[:, n2 * QG:(n2 + 1) * QG])

    # ---- emission schedule: ascending groups, just-in-time projections ----
    # minimal upfront work (attention group 0 only needs v blocks 0..3 and
    # the j=0 q/k columns), so the ACT exp stream starts ~7us earlier and
    # overlaps the remaining projections
    emit_v_proj(0)
    emit_v_proj(1)
    emit_qk_proj(0, 0)
    emit_qk_proj(1, 0)

    # per group: independent projection fillers and out-projection fillers.
    # Out-projections depend on the previous groups' deferred normalization,
    # so only a projection may be scheduled before the norm closure.
    group_proj = {
        0: [lambda: emit_qk_proj(0, 1), lambda: emit_qk_proj(1, 1),
            lambda: emit_v_proj(2), lambda: emit_v_proj(3),
            lambda: emit_v_proj(4), lambda: emit_v_proj(5)],
        1: [lambda: emit_qk_proj(0, 2), lambda: emit_qk_proj(1, 2),
            lambda: emit_v_proj(6), lambda: emit_v_proj(7)],
        2: [lambda: emit_qk_proj(0, 3), lambda: emit_qk_proj(1, 3)],
        3: [],
    }
    group_ops = {
        0: [],
        1: [lambda m=m: emit_outproj(m) for m in range(0, 4)],
        2: [lambda m=m: emit_outproj(m) for m in range(4, 8)],
        3: [lambda m=m: emit_outproj(m) for m in range(8, 12)],
    }

    def sched(proj, ops, nb):
        # [first proj] [deferred norm] [rest of projs] [outprojs]
        f = list(proj)
        if nb is not None:
            f.insert(1 if f else 0, nb)
        return f + list(ops)

    nb = None
    for g in range(NQG):
        proj = group_proj[g]
        ops = group_ops[g]
        ha, hb = (len(proj) + 1) // 2, (len(ops) + 1) // 2
        nb0 = emit_attention_group(0, g, sched(proj[:ha], ops[:hb], nb))
        nb = emit_attention_group(1, g, sched(proj[ha:], ops[hb:], nb0))
    nb()
    # final 4 blocks: all PSUM pools are free once attention ends (6 slots),
    # so open the 8 out-proj chains back-to-back and finish the oldest
    # just-in-time as slots recycle — no per-block copy-drain gaps
    slots = [(pjpool, "pj"), (spool, "sp"), (spool, "sp"),
             (cxpool, "cx"), (cxpool, "cx"), (cxpool, "cx"),
             (pjpool, "pj"), (spool, "sp")]
    ots = {m: outpool.tile([P, D], BF16, tag="ot", name="ot")
           for m in range(12, 16)}
    opened = []

    def finish(m, n2, ps):
        # alternate ACT/DVE so the tail copies drain as two parallel streams
        if n2 == 0:
            nc.scalar.copy(ots[m][:, n2 * QG:(n2 + 1) * QG], ps[:])
        else:
            nc.vector.tensor_copy(ots[m][:, n2 * QG:(n2 + 1) * QG], ps[:])
        nc.sync.dma_start(
            out=out[m * P:(m + 1) * P, n2 * QG:(n2 + 1) * QG],
            in_=ots[m][:, n2 * QG:(n2 + 1) * QG])

    units = [(m, n2) for m in range(12, 16) for n2 in range(2)]
    for u, (m, n2) in enumerate(units):
        if u >= 6:
            finish(*opened.pop(0))
        pool, tag = slots[u]
        ps = pool.tile([P, QG], F32, tag=tag, name="pj")
        for kc in range(2):
            nc.tensor.matmul(
                ps[:],
                lhsT=ctxT[kc][:, m * P:(m + 1) * P],
                rhs=wo_sb[:, kc, n2 * QG:(n2 + 1) * QG],
                start=(kc == 0),
                stop=(kc == 1),
            )
        opened.append((m, n2, ps))
    while opened:
        finish(*opened.pop(0))


def build_nc():
    from contextlib import ExitStack

    nc = bacc.Bacc()
    io = {
        "xT": nc.dram_tensor("xT", [D, S], BF16, kind="ExternalInput").ap(),
        "wq": nc.dram_tensor("wq", [P, KC * DHC], BF16, kind="ExternalInput").ap(),
        "wk": nc.dram_tensor("wk", [P, KC * DHC], BF16, kind="ExternalInput").ap(),
        "wv": nc.dram_tensor("wv", [P, KC * DHC], BF16, kind="ExternalInput").ap(),
        "wo": nc.dram_tensor("wo", [P, 2 * D], BF16, kind="ExternalInput").ap(),
        "out": nc.dram_tensor("out", [S, D], BF16, kind="ExternalOutput").ap(),
    }
    with tile.TileContext(nc) as tc:
        with ExitStack() as ctx:
            _build_body(ctx, tc, io)
    nc.finalize()
    return nc


_NC = None


def _get_nc():
    global _NC
    if _NC is None:
        _NC = build_nc()
    return _NC


def _pack_w(w, sl):
    # [1024, 256] col-slice -> [128, 8*256] (chunk k at cols k*256..)
    bf = ml_dtypes.bfloat16
    w = np.asarray(w, np.float32)[:, sl] if sl is not None else np.asarray(w, np.float32)
    w = w.reshape(KC, P, DHC).transpose(1, 0, 2).reshape(P, KC * DHC)
    return np.ascontiguousarray(w).astype(bf)


def make_in_maps(x, Wq, Wk, Wv, Wo):
    bf = ml_dtypes.bfloat16
    x = np.asarray(x, dtype=np.float32)
    in_maps = []
    xTs = [np.ascontiguousarray(x[b].T).astype(bf) for b in range(B)]
    for c in range(NCORES):
        b, g = divmod(c, 4)
        sl = slice(DHC * g, DHC * (g + 1))
        wo_sl = np.asarray(Wo, np.float32)[sl, :]  # [256, 1024]
        wo_pk = wo_sl.reshape(2, P, D).transpose(1, 0, 2).reshape(P, 2 * D)
        in_maps.append({
            "xT": xTs[b],
            "wq": _pack_w(Wq, sl),
            "wk": _pack_w(Wk, sl),
            "wv": _pack_w(Wv, sl),
            "wo": np.ascontiguousarray(wo_pk).astype(bf),
        })
    return in_maps


def run(in_maps, trace=False, **kw):
    return run_bass_kernel_spmd(_get_nc(), in_maps, list(range(NCORES)),
                                trace=trace, **kw)


def kernel(x, Wq, Wk, Wv, Wo, bo):
    res = run(make_in_maps(x, Wq, Wk, Wv, Wo)).results
    bo = np.asarray(bo, np.float32)
    out = np.empty((B, S, D), np.float32)
    for b in range(B):
        acc = res[4 * b]["out"].astype(np.float32)
        for g in range(1, 4):
            acc = acc + res[4 * b + g]["out"].astype(np.float32)
        out[b] = acc + bo[None, :]
    return out


# revision 68
# speedup vs baseline: 1.0111x; 1.0111x over previous
"""Multi-head causal attention (B=2, S=2048, D=1024, H=16, hd=64) on 8 trn2 cores.

Sharding: core c handles batch b = c//4 and head-group g = c%4 (heads 4g..4g+4,
d-slice 256g..256g+256 of the QKV projections / Wo rows).  Each core computes a
partial out-projection [2048, 1024] in bf16; the host sums the 4 head-group
partials per batch in f32 and adds the bias.

Per-core kernel (all matmuls bf16, accumulate f32 in PSUM):
  qT/kT = (x @ Wq/k)^T computed directly as [256, 2048] via lhsT=W chunks.
  v     = x @ Wv in natural [seq, head, 66] layout (col 64 = 1.0 so the
          attention rowsum falls out of the ctx matmul; col 65 = 0 pad).
  S^T   = k_h @ q_h^T  [kpos, qpos] tiles, both heads of a pair concurrently
          via PE row tiling; exp via ACT (scale=1/8) PSUM->SBUF; causal = skip
          invalid column blocks + triangular bf16 mask on diagonal blocks.
  ctx~T = v'_h^T @ expS^T accumulated over kpos blocks -> [66, 512] PSUM
          (row 64 = softmax denominator).
  norm:  rowsums of both heads copied into one [2,512] tile (ACT+DVE), one
         fast-reciprocal, bf16 downcast, then a K=2 PE matmul against a
         selector constant broadcasts 1/rowsum to [128, 512] PSUM; DVE
         multiplies yield normalized ctx~T (no DRAM bounce round trips).
  out  += ctx~T^T @ Wo rows, stored bf16 per 128-row block as soon as its
         column group's attention is done.

Schedule: ascending q-column groups with just-in-time projections: v/qk(j=0)
projections first, then per group g: attention(pair0, g), attention(pair1, g)
with later projections and the previous group's out-projection interleaved as
PE filler between kb steps.  The attention kb loop is software-pipelined
(scores(kb+1) and a filler are emitted between exp(kb) and ctx(kb)) so the
in-order PE never waits on ACT.  Normalization part B (PE broadcast + DVE
multiplies) is deferred into the next group-half's filler stream, hiding the
reciprocal chain latency.  The final four out-proj blocks cycle their PSUM
chains across all three pools (free once attention ends) to hide copy drain.
Inputs arrive as staged DMAs (weights prepacked host-side into [128, 2048]
row-major tiles; x in column quarters) so the PE starts ~12us in; output is
stored bf16 in half-block stores spread across the back half of the kernel.
"""

import sys

import numpy as np

for _p in ("/opt/trn_rl_repo",):
    if _p not in sys.path:
        sys.path.insert(0, _p)

import ml_dtypes

import concourse.bass as bass
import concourse.mybir as mybir
import concourse.tile as tile
from concourse import bacc
from concourse.bass_utils import run_bass_kernel_spmd
from concourse.masks import make_upper_triangular

BF16 = mybir.dt.bfloat16
F32 = mybir.dt.float32

B, S, D, H, HD = 2, 2048, 1024, 16, 64
NCORES = 8
HPC = 4          # heads per core
DHC = HPC * HD   # 256: d-slice per core
P = 128
SB = S // P      # 16 seq blocks
KC = D // P      # 8 contraction chunks for projections
QG = 512         # q column group width
NQG = S // QG    # 4
VW = HD + 2      # 66: v cols per head (64 data + ones + pad; even M for PE)


def _build_body(ctx, tc, io):
    nc = tc.nc
    xT, wq, wk, wv, wo, out = (
        io["xT"], io["wq"], io["wk"], io["wv"], io["wo"], io["out"],
    )

    consts = ctx.enter_context(tc.tile_pool(name="consts", bufs=1))
    persist = ctx.enter_context(tc.tile_pool(name="persist", bufs=1))
    spool = ctx.enter_context(tc.tile_pool(name="spsum", bufs=2, space="PSUM"))
    cxpool = ctx.enter_context(tc.tile_pool(name="cxpsum", bufs=3, space="PSUM"))
    pjpool = ctx.enter_context(tc.tile_pool(name="pjpsum", bufs=1, space="PSUM"))
    espool = ctx.enter_context(tc.tile_pool(name="es", bufs=6))
    nrmpool = ctx.enter_context(tc.tile_pool(name="nrm", bufs=4))
    outpool = ctx.enter_context(tc.tile_pool(name="outsb", bufs=3))

    # triangular keep-mask for diagonal blocks: tri[i, j] = 1.0 iff j >= i
    tri = consts.tile([P, P], BF16, tag="tri", name="tri")
    make_upper_triangular(nc, tri[:], val=1.0, diag=True)
    # tri[0:1, 0:HD] doubles as the all-ones [1, 64] vector for K=1
    # partition-broadcast matmuls in the softmax normalization.

    def emit_pe_warmup():
        # dummy matmuls on the tri constant during the input-DMA wait: the
        # PE DVFS ramp (0.65 -> 2.4 GHz over ~3us of continuous execution)
        # completes before the first real projection chains
        wps = pjpool.tile([P, QG], F32, tag="pj", name="warm")
        for _ in range(12):
            nc.tensor.matmul(wps[:, 0:P], lhsT=tri[:], rhs=tri[:],
                             start=True, stop=True)

    # ---- input tiles + staged DMA issue order ----
    wq_sb = persist.tile([P, KC, DHC], BF16, tag="wq", name="wq")
    wk_sb = persist.tile([P, KC, DHC], BF16, tag="wk", name="wk")
    wv_sb = persist.tile([P, KC, DHC], BF16, tag="wv", name="wv")
    wo_sb = persist.tile([P, 2, D], BF16, tag="wo", name="wo")
    xt = [persist.tile([P, S], BF16, tag=f"xt{k}", name=f"xt{k}")
          for k in range(KC)]

    HK = KC // 2  # weight half: 4 k-chunks
    HX = S // 2   # x column half
    QX = HX // 2  # x column quarter

    def dma_x(k, q):
        nc.sync.dma_start(out=xt[k][:, q * QX:(q + 1) * QX],
                          in_=xT[k * P:(k + 1) * P, q * QX:(q + 1) * QX])

    # interleave wv chunk-pairs with the x chunks they gate so the first
    # v-projection's 8-chunk PSUM chain starts after ~3 DMAs land
    for h in range(4):
        nc.sync.dma_start(out=wv_sb[:, 2 * h:2 * (h + 1), :],
                          in_=wv[:, 2 * h * DHC:2 * (h + 1) * DHC])
        dma_x(2 * h, 0)
        dma_x(2 * h + 1, 0)
    for w_sb, dram in ((wq_sb, wq), (wk_sb, wk)):
        for h in range(2):
            nc.sync.dma_start(out=w_sb[:, h * HK:(h + 1) * HK, :],
                              in_=dram[:, h * HK * DHC:(h + 1) * HK * DHC])
    for k in range(KC):
        dma_x(k, 1)
    for q in range(2, 4):
        for k in range(KC):
            dma_x(k, q)
    for h in range(2):
        nc.sync.dma_start(out=wo_sb[:, h, :], in_=wo[:, h * D:(h + 1) * D])

    # persistent tensors
    v_sb = [persist.tile([P, HPC, VW], BF16, tag=f"v{s}", name=f"v{s}")
            for s in range(SB)]
    qt = [persist.tile([P, S], BF16, tag=f"qt{i}", name=f"qt{i}") for i in range(2)]
    kt = [persist.tile([P, S], BF16, tag=f"kt{i}", name=f"kt{i}") for i in range(2)]
    ctxT = [persist.tile([P, S], BF16, tag=f"ctxT{i}", name=f"ctxT{i}")
            for i in range(2)]

    # ---- emission helpers ----
    def emit_v_proj(sv):
        # two seq blocks (2*sv, 2*sv+1) -> v natural layout
        ps = spool.tile([P, 2, QG], F32, tag="sp", name="sp")
        for par in range(2):
            s = 2 * sv + par
            for k in range(KC):
                nc.tensor.matmul(
                    ps[:, par, 0:DHC],
                    lhsT=xt[k][:, s * P:(s + 1) * P],
                    rhs=wv_sb[:, k, :],
                    start=(k == 0),
                    stop=(k == KC - 1),
                )
            src_ap = ps[:, par, 0:DHC].rearrange("p (h d) -> p h d", h=HPC)
            nc.vector.tensor_copy(v_sb[s][:, :, 0:HD], src_ap)
            nc.vector.memset(v_sb[s][:, :, HD:VW], 1.0)
            nc.vector.memset(v_sb[s][:, :, HD + 1:VW], 0.0)

    def emit_qk_proj(pair, j):
        # q and k projections for d-chunk `pair`, q column group j
        for w_sb, dst in ((wq_sb, qt), (wk_sb, kt)):
            ps = pjpool.tile([P, QG], F32, tag="pj", name="pj")
            for k in range(KC):
                nc.tensor.matmul(
                    ps[:],
                    lhsT=w_sb[:, k, pair * P:(pair + 1) * P],
                    rhs=xt[k][:, j * QG:(j + 1) * QG],
                    start=(k == 0),
                    stop=(k == KC - 1),
                )
            nc.vector.tensor_copy(dst[pair][:, j * QG:(j + 1) * QG], ps[:])

    def emit_attention_group(pair, g, fillers):
        # fillers: list of zero-arg closures; one is popped and emitted after
        # each kb step to keep independent PE work between dependent steps.
        # The kb loop is software-pipelined: scores(kb+1) is emitted before
        # ctx(kb) so the PE runs scores while ACT computes exp(kb).
        cxs = [cxpool.tile([VW, QG], F32, tag="cx", name="cx") for _ in range(2)]
        nkb = 4 * g + 4

        def c0_of(kb):
            return P * (kb - 4 * g) if kb >= 4 * g else 0

        def emit_scores(kb):
            c0 = c0_of(kb)
            sp_t = spool.tile([P, 2, QG], F32, tag="sp", name="sp")
            for hh in range(2):
                nc.tensor.matmul(
                    sp_t[:, hh, c0:QG],
                    lhsT=kt[pair][hh * HD:(hh + 1) * HD, kb * P:(kb + 1) * P],
                    rhs=qt[pair][hh * HD:(hh + 1) * HD, g * QG + c0:(g + 1) * QG],
                    start=True,
                    stop=True,
                )
            return sp_t

        sp_next = emit_scores(0)
        for kb in range(nkb):
            c0 = c0_of(kb)
            sp_t = sp_next
            es_t = espool.tile([P, 2, QG], BF16, tag="es", name="es")
            nc.scalar.activation(
                es_t[:, :, c0:QG], sp_t[:, :, c0:QG],
                mybir.ActivationFunctionType.Exp, scale=0.125,
            )
            if kb + 1 < nkb:
                sp_next = emit_scores(kb + 1)
            if fillers:
                fillers.pop(0)()
            if kb == 0 and fillers:
                # second filler before the first ctx: covers the deferred
                # normalization's reciprocal-chain latency so ctx(0) doesn't
                # stall on the PSUM bank its predecessor frees
                fillers.pop(0)()
            if kb >= 4 * g:
                dst = es_t[:, :, c0:c0 + P]
                t_ap = tri[:]
                tri_b = bass.AP(t_ap.tensor, t_ap.offset,
                                [t_ap.ap[0], [0, 2], t_ap.ap[1]])
                nc.vector.tensor_mul(dst, dst, tri_b)
            for hh in range(2):
                h = 2 * pair + hh
                nc.tensor.matmul(
                    cxs[hh][:, c0:QG],
                    lhsT=v_sb[kb][:, h, :],
                    rhs=es_t[:, hh, c0:QG],
                    start=(kb == 0),
                    stop=(kb == nkb - 1),
                )
        while fillers:
            fillers.pop(0)()
        # softmax normalization part A (inline): per-head rowsum -> reciprocal
        # -> bf16 downcast, all on DVE ([1, QG] ops pipeline per head and the
        # ACT queue stays clear for the next group's exps).  Separate
        # offset-0 tiles per head: custom DVE ops need zero-offset APs.
        rcbs = []
        for hh in range(2):
            rs1 = nrmpool.tile([1, QG], F32, tag=f"rs{hh}", name="rs")
            rc1 = nrmpool.tile([1, QG], F32, tag=f"rc{hh}", name="rc")
            rcb1 = nrmpool.tile([1, QG], BF16, tag=f"rcb{hh}", name="rcb")
            nc.vector.tensor_copy(rs1[:], cxs[hh][HD:HD + 1, :])
            nc.vector.reciprocal_approx_fast(rc1[:], rs1[:])
            nc.vector.tensor_copy(rcb1[:], rc1[:])
            rcbs.append(rcb1)

        def norm_b():
            # part B (deferred into the next group's PE stream): K=1 PE
            # broadcasts of 1/rowsum + normalized fp-copy of ctx~T
            rb = pjpool.tile([P, QG], F32, tag="pj", name="rb")
            for hh in range(2):
                nc.tensor.matmul(
                    rb[hh * HD:(hh + 1) * HD, :],
                    lhsT=tri[0:1, 0:HD],
                    rhs=rcbs[hh][0:1, :],
                    start=True,
                    stop=True,
                )
            rbs = nrmpool.tile([P, QG], F32, tag="rbs", name="rbs")
            nc.vector.tensor_copy(rbs[:], rb[:])
            for hh in range(2):
                nc.vector.tensor_mul(
                    ctxT[pair][hh * HD:(hh + 1) * HD, g * QG:(g + 1) * QG],
                    cxs[hh][0:HD, :],
                    rbs[hh * HD:(hh + 1) * HD, :],
                )

        return norm_b

    def emit_outproj(m, pools=None):
        # pools: optional per-half psum pool/tag overrides; the final blocks
        # cycle over all pools (free once attention is done) so consecutive
        # chains don't serialize on one bank's copy drain.
        ot = outpool.tile([P, D], BF16, tag="ot", name="ot")
        for n2 in range(2):
            if pools is None:
                ps = pjpool.tile([P, QG], F32, tag="pj", name="pj")
            else:
                pool, tag = pools[n2]
                ps = pool.tile([P, QG], F32, tag=tag, name="pj")
            for kc in range(2):
                nc.tensor.matmul(
                    ps[:],
                    lhsT=ctxT[kc][:, m * P:(m + 1) * P],
                    rhs=wo_sb[:, kc, n2 * QG:(n2 + 1) * QG],
                    start=(kc == 0),
                    stop=(kc == 1),
                )
            if pools is None:
                nc.vector.tensor_copy(ot[:, n2 * QG:(n2 + 1) * QG], ps[:])
            else:
                # final blocks: ACT is idle at the tail, keep DVE clear
                nc.scalar.copy(ot[:, n2 * QG:(n2 + 1) * QG], ps[:])
            nc.sync.dma_start(
                out=out[m * P:(m + 1) * P, n2 * QG:(n2 + 1) * QG],
                in_=ot[:, n2 * QG:(n2 + 1) * QG])

    # ---- emission schedule: ascending groups, just-in-time projections ----
    # minimal upfront work (attention group 0 only needs v blocks 0..3 and
    # the j=0 q/k columns), so the ACT exp stream starts ~7us earlier and
    # overlaps the remaining projections
    emit_pe_warmup()
    emit_v_proj(0)
    emit_v_proj(1)
    emit_qk_proj(0, 0)
    emit_qk_proj(1, 0)

    # per group: independent projection fillers and out-projection fillers.
    # Out-projections depend on the previous groups' deferred normalization,
    # so only a projection may be scheduled before the norm closure.
    group_proj = {
        0: [lambda: emit_qk_proj(0, 1), lambda: emit_qk_proj(1, 1),
            lambda: emit_v_proj(2), lambda: emit_v_proj(3),
            lambda: emit_v_proj(4), lambda: emit_v_proj(5)],
        1: [lambda: emit_qk_proj(0, 2), lambda: emit_qk_proj(1, 2),
            lambda: emit_v_proj(6), lambda: emit_v_proj(7)],
        2: [lambda: emit_qk_proj(0, 3), lambda: emit_qk_proj(1, 3)],
        3: [],
    }
    group_ops = {
        0: [],
        1: [lambda m=m: emit_outproj(m) for m in range(0, 4)],
        2: [lambda m=m: emit_outproj(m) for m in range(4, 8)],
        3: [lambda m=m: emit_outproj(m) for m in range(8, 12)],
    }

    def sched(proj, ops, nb):
        # [first proj] [deferred norm] [rest of projs] [outprojs]
        f = list(proj)
        if nb is not None:
            f.insert(1 if f else 0, nb)
        return f + list(ops)

    nb = None
    for g in range(NQG):
        proj = group_proj[g]
        ops = group_ops[g]
        ha, hb = (len(proj) + 1) // 2, (len(ops) + 1) // 2
        nb0 = emit_attention_group(0, g, sched(proj[:ha], ops[:hb], nb))
        nb = emit_attention_group(1, g, sched(proj[ha:], ops[hb:], nb0))
    nb()
    # final 4 blocks: all PSUM pools are free once attention ends (6 slots),
    # so open the 8 out-proj chains back-to-back and finish the oldest
    # just-in-time as slots recycle — no per-block copy-drain gaps
    slots = [(pjpool, "pj"), (spool, "sp"), (spool, "sp"),
             (cxpool, "cx"), (cxpool, "cx"), (cxpool, "cx"),
             (pjpool, "pj"), (spool, "sp")]
    ots = {m: outpool.tile([P, D], BF16, tag="ot", name="ot")
           for m in range(12, 16)}
    opened = []

    def finish(m, n2, ps):
        # alternate ACT/DVE so the tail copies drain as two parallel streams
        if n2 == 0:
            nc.scalar.copy(ots[m][:, n2 * QG:(n2 + 1) * QG], ps[:])
        else:
            nc.vector.tensor_copy(ots[m][:, n2 * QG:(n2 + 1) * QG], ps[:])
        # the very last block's stores split across twice the queues: the
        # terminal store drain is latency- (not descriptor-) bound
        nstore = 2 if m == 15 else 1
        for si in range(nstore):
            c0s = n2 * QG + si * (QG // nstore)
            c1s = c0s + QG // nstore
            nc.sync.dma_start(out=out[m * P:(m + 1) * P, c0s:c1s],
                              in_=ots[m][:, c0s:c1s])

    units = [(m, n2) for m in range(12, 16) for n2 in range(2)]
    for u, (m, n2) in enumerate(units):
        if u >= 6:
            finish(*opened.pop(0))
        pool, tag = slots[u]
        ps = pool.tile([P, QG], F32, tag=tag, name="pj")
        for kc in range(2):
            nc.tensor.matmul(
                ps[:],
                lhsT=ctxT[kc][:, m * P:(m + 1) * P],
                rhs=wo_sb[:, kc, n2 * QG:(n2 + 1) * QG],
                start=(kc == 0),
                stop=(kc == 1),
            )
        opened.append((m, n2, ps))
    while opened:
        finish(*opened.pop(0))


def build_nc():
    from contextlib import ExitStack

    nc = bacc.Bacc()
    io = {
        "xT": nc.dram_tensor("xT", [D, S], BF16, kind="ExternalInput").ap(),
        "wq": nc.dram_tensor("wq", [P, KC * DHC], BF16, kind="ExternalInput").ap(),
        "wk": nc.dram_tensor("wk", [P, KC * DHC], BF16, kind="ExternalInput").ap(),
        "wv": nc.dram_tensor("wv", [P, KC * DHC], BF16, kind="ExternalInput").ap(),
        "wo": nc.dram_tensor("wo", [P, 2 * D], BF16, kind="ExternalInput").ap(),
        "out": nc.dram_tensor("out", [S, D], BF16, kind="ExternalOutput").ap(),
    }
    with tile.TileContext(nc) as tc:
        with ExitStack() as ctx:
            _build_body(ctx, tc, io)
    nc.finalize()
    return nc


_NC = None


def _get_nc():
    global _NC
    if _NC is None:
        _NC = build_nc()
    return _NC


def _pack_w(w, sl):
    # [1024, 256] col-slice -> [128, 8*256] (chunk k at cols k*256..)
    bf = ml_dtypes.bfloat16
    w = np.asarray(w, np.float32)[:, sl] if sl is not None else np.asarray(w, np.float32)
    w = w.reshape(KC, P, DHC).transpose(1, 0, 2).reshape(P, KC * DHC)
    return np.ascontiguousarray(w).astype(bf)


def make_in_maps(x, Wq, Wk, Wv, Wo):
    bf = ml_dtypes.bfloat16
    x = np.asarray(x, dtype=np.float32)
    in_maps = []
    xTs = [np.ascontiguousarray(x[b].T).astype(bf) for b in range(B)]
    for c in range(NCORES):
        b, g = divmod(c, 4)
        sl = slice(DHC * g, DHC * (g + 1))
        wo_sl = np.asarray(Wo, np.float32)[sl, :]  # [256, 1024]
        wo_pk = wo_sl.reshape(2, P, D).transpose(1, 0, 2).reshape(P, 2 * D)
        in_maps.append({
            "xT": xTs[b],
            "wq": _pack_w(Wq, sl),
            "wk": _pack_w(Wk, sl),
            "wv": _pack_w(Wv, sl),
            "wo": np.ascontiguousarray(wo_pk).astype(bf),
        })
    return in_maps


def run(in_maps, trace=False, **kw):
    return run_bass_kernel_spmd(_get_nc(), in_maps, list(range(NCORES)),
                                trace=trace, **kw)


def kernel(x, Wq, Wk, Wv, Wo, bo):
    res = run(make_in_maps(x, Wq, Wk, Wv, Wo)).results
    bo = np.asarray(bo, np.float32)
    out = np.empty((B, S, D), np.float32)
    for b in range(B):
        acc = res[4 * b]["out"].astype(np.float32)
        for g in range(1, 4):
            acc = acc + res[4 * b + g]["out"].astype(np.float32)
        out[b] = acc + bo[None, :]
    return out
